# revision 24
# baseline (speedup 1.0000x reference)
"""C2Q attention kernel for 8 TRN2 NeuronCores — ragged-aware.

Math (per batch):
    score  = (o_c @ W @ o_q.T + (o_c @ b) 1^T) / sqrt(H)   [Tc, Tq]
    prob   = softmax_j(score masked at j>=q_len)
    out    = (prob * (i < c_len)) @ o_q                     [Tc, H]

Two exploits make the device program smaller than the dense math:
  * softmax is invariant to a per-row constant, so the bias term
    (o_c@b)1^T cancels exactly -> never computed.
  * by associativity the H x H projection can hit either side:
    (o_c @ W) @ o_q.T  or  o_c @ (W @ o_q.T).  Each slot picks the
    side with the shorter length, so the Linear costs 64*min(q,c)
    PE-rows instead of 64*Tq.

Ragged scheduling: the program is compiled AT RUNTIME for the actual
lengths.  The 32 batches are grouped into 4 slots x 8 cores so that
each slot's compile-time shape (q~, c~) = componentwise max over its 8
batches (grouping chosen by local search to minimize total PE rows).
All cores run the identical 4-slot program on their own batch of each
slot -> SPMD holds, but ~25% of the dense FLOPs are never issued.

Device layout per slot (everything lands K-on-partitions, no on-chip
transposes):
    proj   = 8 psum groups, free dim = min(q~,c~)     [128, 8*L] f16
    e[j,i] = exp(score/32 + qbias[j]) per j-tile      [<=128, c~] f16
             (qbias in {0,-60000} -> masked j gives exactly 0)
    ctx    = e.T @ [1 | o_q] in 3 free-blocks of ~342; the ones column
             makes d[i] = sum_j e[j,i] land in psum col 0, already
             per-partition -> reciprocal feeds the eviction scale.
c_len masking is host-side (only rows < c_len are copied out).
"""

import os
import sys

import numpy as np

if "/opt/trn_rl_repo" not in sys.path:
    sys.path.insert(0, "/opt/trn_rl_repo")

B, Tc, Tq, H = 32, 512, 512, 1024
N_CORES = 8
N_SLOTS = B // N_CORES  # 4
KT = H // 128  # contraction tiles over features (8)
OW = 1032  # oqN slab block width: [ones | h0..h1023] padded
SCALE = 1.0 / 32.0  # 1/sqrt(H)
NEG16 = np.float16(-60000.0)  # exp(x - 60000) == 0 exactly in fp32

CTX_BLOCKS = [(0, 342), (342, 684), (684, 1025)]  # cols of [1 | h...]


def _r16(x: int) -> int:
    return -(-int(x) // 16) * 16


def _rows(qm: int, cm: int) -> int:
    """PE row cost of one slot with shape (q~, c~)."""
    q, c = _r16(qm), _r16(cm)
    jt, it = -(-q // 128), -(-c // 128)
    return 64 * min(q, c) + 8 * jt * c + it * jt * 1025


def _group_batches(q_len, c_len):
    """Partition 32 batches into 4 groups of 8 minimizing slot-max cost."""
    import random

    rng = random.Random(12345)
    n = len(q_len)

    def total(groups):
        return sum(
            _rows(max(q_len[i] for i in g), max(c_len[i] for i in g))
            for g in groups
        )

    best_t, best_g = None, None
    for trial in range(12):
        order = sorted(
            range(n), key=lambda i: -(q_len[i] * 1024 + c_len[i])
        ) if trial == 0 else rng.sample(range(n), n)
        groups = [order[i * 8 : (i + 1) * 8] for i in range(N_SLOTS)]
        cur = total(groups)
        for _ in range(20000):
            g1, g2 = rng.sample(range(N_SLOTS), 2)
            i1, i2 = rng.randrange(8), rng.randrange(8)
            groups[g1][i1], groups[g2][i2] = groups[g2][i2], groups[g1][i1]
            t = total(groups)
            if t <= cur:
                cur = t
            else:
                groups[g1][i1], groups[g2][i2] = groups[g2][i2], groups[g1][i1]
        if best_t is None or cur < best_t:
            best_t, best_g = cur, [list(g) for g in groups]
    # order slots by descending cost: the big slot rides out the DMA ramp
    # (most PE work per input byte), the small slot gives a short tail
    costs = [
        _rows(max(q_len[i] for i in g), max(c_len[i] for i in g))
        for g in best_g
    ]
    order = sorted(range(N_SLOTS), key=lambda s: -costs[s])
    return [best_g[s] for s in order]


def _build_program(slots):
    """slots: list of dicts with qt, ct, jt, it, side ('q'|'c')."""
    import concourse.bacc as bacc
    import concourse.mybir as mybir
    import concourse.tile as tile

    f32 = mybir.dt.float32
    f16 = mybir.dt.float16
    nc = bacc.Bacc("TRN2", debug=False)

    need_q = any(s["side"] == "q" for s in slots)
    need_c = any(s["side"] == "c" for s in slots)

    wtq_d = nc.declare_dram_parameter("wtq", [128, 4, KT, 256], f16, isOutput=False) if need_q else None
    wtc_d = nc.declare_dram_parameter("wtc", [128, 4, KT, 256], f16, isOutput=False) if need_c else None
    oqT_d, ocT_d, oqN_d, out_d = [], [], [], []
    for s, sl in enumerate(slots):
        qt, ct, jt, it = sl["qt"], sl["ct"], sl["jt"], sl["it"]
        oqT_d.append(nc.declare_dram_parameter(f"oqT{s}", [128, KT * qt + jt], f16, isOutput=False))
        ocT_d.append(nc.declare_dram_parameter(f"ocT{s}", [128, KT * ct], f16, isOutput=False))
        oqN_d.append(nc.declare_dram_parameter(f"oqN{s}", [128, jt * OW], f16, isOutput=False))
        out_d.append(nc.declare_dram_parameter(f"out{s}", [ct, H], f16, isOutput=True))

    with tile.TileContext(nc) as tc:
        with (
            tc.tile_pool(name="const", bufs=1) as cpool,
            tc.tile_pool(name="inp", bufs=2) as ipool,
            tc.tile_pool(name="work", bufs=1) as wpool,
            tc.tile_pool(name="outp", bufs=3) as opool,
            tc.tile_pool(name="ps_acc", bufs=2, space="PSUM") as ps_acc,
            tc.tile_pool(name="ps_ctx", bufs=2, space="PSUM") as ps_ctx,
        ):
            wtq = cpool.tile([128, 4, KT, 256], f16, tag="wtq", name="wtq") if need_q else None
            wtc = cpool.tile([128, 4, KT, 256], f16, tag="wtc", name="wtc") if need_c else None

            # per-slot state carried between emission phases
            st = [dict() for _ in slots]

            def dma_inputs(s, fine):
                sl = slots[s]
                qt, ct, jt = sl["qt"], sl["ct"], sl["jt"]
                oqT = ipool.tile([128, KT * qt + jt], f16, tag="oqT", name=f"oqT_{s}")
                ocT = ipool.tile([128, KT * ct], f16, tag="ocT", name=f"ocT_{s}")
                oqN = ipool.tile([128, jt * OW], f16, tag="oqN", name=f"oqN_{s}")
                st[s].update(oqT=oqT, ocT=ocT, oqN=oqN)
                if fine:
                    # ramp schedule: oqT k-slices stream in order while the
                    # wt slab arrives per o-BLOCK (3D AP, one desc each), so
                    # lin group o can start as soon as wt-o + early k-slices
                    # land; the paired lin0 emission consumes each k-slice
                    # twice, matching the DMA arrival rate
                    w_slab = wtq if sl["side"] == "q" else wtc
                    w_d = wtq_d if sl["side"] == "q" else wtc_d
                    L = qt if sl["side"] == "q" else ct
                    mov, mov_d = (oqT, oqT_d[s]) if sl["side"] == "q" else (ocT, ocT_d[s])
                    oth, oth_d = (ocT, ocT_d[s]) if sl["side"] == "q" else (oqT, oqT_d[s])
                    # pair0 of lin0 needs ALL oqT k-slices but only the
                    # first wt o-block: stream those first, the rest after
                    for k in range(KT):
                        lo, hi = k * L, (k + 1) * L
                        if k == KT - 1 and mov is oqT:
                            hi += jt  # qb bias columns ride with the last slice
                        nc.sync.dma_start(out=mov[:, lo:hi], in_=mov_d[:, lo:hi])
                        if k == 0:
                            nc.sync.dma_start(out=w_slab[:, 0], in_=w_d[:, 0])
                    for ob in range(1, 4):
                        nc.sync.dma_start(out=w_slab[:, ob], in_=w_d[:, ob])
                        if ob == 2:
                            # the score-side slab is needed ~12us in; for
                            # side-c it also carries the qb bias columns
                            nc.sync.dma_start(out=oth, in_=oth_d[:, :])
                    other_w = wtc if (sl["side"] == "q" and need_c) else (wtq if (sl["side"] == "c" and need_q) else None)
                    other_wd = wtc_d if sl["side"] == "q" else wtq_d
                    if other_w is not None:
                        nc.sync.dma_start(out=other_w[:, :2], in_=other_wd[:, :2])
                        nc.sync.dma_start(out=other_w[:, 2:], in_=other_wd[:, 2:])
                else:
                    nc.sync.dma_start(out=oqT, in_=oqT_d[s][:, :])
                    nc.sync.dma_start(out=ocT, in_=ocT_d[s][:, :])
                # all DMA stays on the sync queue: waking the GpSimd queue
                # costs the PE its boost p-state (measured 2.37 -> 2.0 GHz)
                nc.sync.dma_start(out=oqN, in_=oqN_d[s][:, :])

            def linear_gen(s):
                """Yield one emission step (matmul / eviction) at a time so
                ctx(s-1) can interleave them into its eviction bubbles."""
                sl = slots[s]
                qt, ct = sl["qt"], sl["ct"]
                L = qt if sl["side"] == "q" else ct
                w_slab = wtq if sl["side"] == "q" else wtc
                mov = st[s]["oqT"] if sl["side"] == "q" else st[s]["ocT"]
                proj = wpool.tile([128, KT * 512], f16, tag="proj", name=f"proj_{s}")
                st[s]["proj"] = proj
                for o in range(KT):
                    ups = ps_acc.tile([128, 512], f32, tag="acc", name=f"ups{o}_{s}")
                    for k in range(KT):
                        nc.tensor.matmul(
                            ups[:, :L],
                            w_slab[:, o // 2, k, (o % 2) * 128 : (o % 2 + 1) * 128],
                            mov[:, k * L : (k + 1) * L],
                            start=(k == 0),
                            stop=(k == KT - 1),
                        )
                        yield
                    nc.vector.tensor_scalar(
                        out=proj[:, o * L : (o + 1) * L],
                        in0=ups[:, :L],
                        scalar1=1.0,
                        scalar2=None,
                        op0=mybir.AluOpType.mult,
                    )

            def linear0():
                """Slot-0 Linear with o-groups in pairs, k-interleaved: each
                arriving oqT k-slice feeds two matmuls, matching the DMA
                arrival rate during the ramp."""
                sl = slots[0]
                qt, ct = sl["qt"], sl["ct"]
                L = qt if sl["side"] == "q" else ct
                w_slab = wtq if sl["side"] == "q" else wtc
                mov = st[0]["oqT"] if sl["side"] == "q" else st[0]["ocT"]
                proj = wpool.tile([128, KT * 512], f16, tag="proj", name="proj_0")
                st[0]["proj"] = proj
                for og in range(0, KT, 2):
                    ups = [
                        ps_acc.tile([128, 512], f32, tag="acc", name=f"ups{og + i}_0")
                        for i in range(2)
                    ]
                    for k in range(KT):
                        for i in range(2):
                            o = og + i
                            nc.tensor.matmul(
                                ups[i][:, :L],
                                w_slab[:, o // 2, k, (o % 2) * 128 : (o % 2 + 1) * 128],
                                mov[:, k * L : (k + 1) * L],
                                start=(k == 0),
                                stop=(k == KT - 1),
                            )
                    for i in range(2):
                        nc.vector.tensor_scalar(
                            out=proj[:, (og + i) * L : (og + i + 1) * L],
                            in0=ups[i][:, :L],
                            scalar1=1.0,
                            scalar2=None,
                            op0=mybir.AluOpType.mult,
                        )

            def drain(gen, n):
                if gen is None:
                    return
                for _ in range(n):
                    if next(gen, StopIteration) is StopIteration:
                        return

            def score(s):
                sl = slots[s]
                qt, ct, jt = sl["qt"], sl["ct"], sl["jt"]
                stat = st[s]["proj"] if sl["side"] == "q" else st[s]["oqT"]
                mov = st[s]["ocT"] if sl["side"] == "q" else st[s]["proj"]
                stat_L = qt  # j-slices always live in qt-wide sections
                mov_L = ct
                qb = st[s]["oqT"][:, KT * qt : KT * qt + jt]
                e_tiles = []
                for t in range(jt):
                    mj = min(128, qt - t * 128)
                    sps = ps_acc.tile([128, 512], f32, tag="acc", name=f"sps{t}_{s}")
                    for o in range(KT):
                        nc.tensor.matmul(
                            sps[:mj, :ct],
                            stat[:, o * stat_L + t * 128 : o * stat_L + t * 128 + mj],
                            mov[:, o * mov_L : (o + 1) * mov_L],
                            start=(o == 0),
                            stop=(o == KT - 1),
                        )
                    e = wpool.tile([128, 512], f16, tag=f"e{t}", name=f"e{t}_{s}")
                    nc.scalar.activation(
                        out=e[:mj, :ct],
                        in_=sps[:mj, :ct],
                        func=mybir.ActivationFunctionType.Exp,
                        bias=qb[:mj, t : t + 1],
                        scale=SCALE,
                    )
                    e_tiles.append(e)
                st[s]["e"] = e_tiles

            def ctx(s, lin):
                """Emit ctx(s); weave next slot's Linear matmuls (lin gen)
                between psum groups so evictions never stall the PE."""
                sl = slots[s]
                qt, ct, jt, it = sl["qt"], sl["ct"], sl["jt"], sl["it"]
                e_tiles, oqN = st[s]["e"], st[s]["oqN"]
                drain(lin, 4)  # cover the last exp's latency
                for ti in range(it):
                    mi = min(128, ct - ti * 128)
                    cps = []
                    for bi, (c0, c1) in enumerate(CTX_BLOCKS):
                        cp = ps_ctx.tile([128, 342], f32, tag=f"ctx{bi}", name=f"cps{ti}{bi}_{s}")
                        for t in range(jt):
                            mj = min(128, qt - t * 128)
                            nc.tensor.matmul(
                                cp[:mi, : c1 - c0],
                                e_tiles[t][:mj, ti * 128 : ti * 128 + mi],
                                oqN[:mj, t * OW + c0 : t * OW + c1],
                                start=(t == 0),
                                stop=(t == jt - 1),
                            )
                        cps.append(cp)
                        drain(lin, 6)
                    r = wpool.tile([128, 1], f32, tag="r", name=f"r{ti}_{s}")
                    nc.vector.reciprocal(out=r[:mi], in_=cps[0][:mi, 0:1])
                    osb = opool.tile([128, H], f16, tag="osb", name=f"osb{ti}_{s}")
                    # all evictions on DVE (keeping ACT/GpSimd quiet protects
                    # the PE boost clock); the lin interleave gives DVE ~6us
                    # per i-tile of slack
                    h0 = 0
                    for bi, (c0, c1) in enumerate(CTX_BLOCKS):
                        s0 = 1 if bi == 0 else 0  # skip the ones column
                        w = (c1 - c0) - s0
                        nc.vector.tensor_scalar(
                            out=osb[:mi, h0 : h0 + w],
                            in0=cps[bi][:mi, s0 : c1 - c0],
                            scalar1=r[:mi],
                            scalar2=None,
                            op0=mybir.AluOpType.mult,
                        )
                        h0 += w
                    nc.sync.dma_start(
                        out=out_d[s][ti * 128 : ti * 128 + mi, :],
                        in_=osb[:mi, :],
                    )

            # PE order: lin0 score0 | ctx0<<lin1 score1 | ctx1<<lin2 score2
            #           | ctx2<<lin3 score3 | ctx3
            dma_inputs(0, fine=True)
            dma_inputs(1, fine=False)
            linear0()
            score(0)
            for s in range(N_SLOTS):
                if s + 2 < N_SLOTS:
                    dma_inputs(s + 2, fine=False)
                lin = linear_gen(s + 1) if s + 1 < N_SLOTS else None
                ctx(s, lin)
                if lin is not None:
                    drain(lin, 1000)  # finish any remaining lin steps
                    score(s + 1)

    nc.compile()
    return nc


def _plan(q_lengths, c_lengths):
    groups = _group_batches(list(map(int, q_lengths)), list(map(int, c_lengths)))
    slots = []
    for g in groups:
        qt = _r16(max(int(q_lengths[i]) for i in g))
        ct = _r16(max(int(c_lengths[i]) for i in g))
        slots.append(
            dict(
                qt=qt, ct=ct,
                jt=-(-qt // 128), it=-(-ct // 128),
                side="q" if qt <= ct else "c",
                batches=list(g),
            )
        )
    return slots


def _host_inputs(o_c, o_q, W, q_lengths, slots):
    """Per-core input maps (host-side sharding + re-layout), all fp16."""
    need_q = any(s["side"] == "q" for s in slots)
    need_c = any(s["side"] == "c" for s in slots)
    maps = [dict() for _ in range(N_CORES)]
    if need_q:
        # wtq[p, ob, k, c] = W[ob*256 + c, k*128 + p]
        wtq = np.ascontiguousarray(
            W.reshape(4, 256, 8, 128).transpose(3, 0, 2, 1)
        ).astype(np.float16)
        for m in maps:
            m["wtq"] = wtq
    if need_c:
        # wtc[p, mb, k, c] = W[k*128 + p, mb*256 + c]
        wtc = np.ascontiguousarray(
            W.reshape(8, 128, 4, 256).transpose(1, 2, 0, 3)
        ).astype(np.float16)
        for m in maps:
            m["wtc"] = wtc

    jidx = np.arange(128)[:, None]  # partition index within a j-tile
    for s, sl in enumerate(slots):
        qt, ct, jt = sl["qt"], sl["ct"], sl["jt"]
        for core, g in enumerate(sl["batches"]):
            oq = o_q[g]  # [Tq, H] f32
            oc = o_c[g]
            # oqT: [p, k*qt + j] = oq[j, k*128+p], + jt qbias columns
            oqT = np.empty((128, KT * qt + jt), np.float16)
            oqT[:, : KT * qt] = (
                oq[:qt].T.reshape(KT, 128, qt).transpose(1, 0, 2).reshape(128, KT * qt)
            )
            ql = int(q_lengths[g])
            tcol = np.arange(jt)[None, :] * 128 + jidx  # [128, jt]
            oqT[:, KT * qt :] = np.where(tcol < ql, np.float16(0.0), NEG16)
            # ocT: [p, k*ct + i] = oc[i, k*128+p]
            ocT = (
                oc[:ct].T.reshape(KT, 128, ct).transpose(1, 0, 2)
                .reshape(128, KT * ct)
            ).astype(np.float16)
            # oqN: per j-tile block [ones | oq rows]
            oqN = np.zeros((128, jt * OW), np.float16)
            for t in range(jt):
                oqN[:, t * OW] = 1.0
                oqN[:, t * OW + 1 : t * OW + 1 + H] = oq[t * 128 : (t + 1) * 128]
            maps[core][f"oqT{s}"] = np.ascontiguousarray(oqT)
            maps[core][f"ocT{s}"] = np.ascontiguousarray(ocT)
            maps[core][f"oqN{s}"] = np.ascontiguousarray(oqN)
    return maps


def kernel(**inputs) -> np.ndarray:
    o_c = np.asarray(inputs["o_c"], dtype=np.float32)
    o_q = np.asarray(inputs["o_q"], dtype=np.float32)
    W = np.asarray(inputs["W"], dtype=np.float32)
    q_lengths = np.asarray(inputs["q_lengths"]).astype(np.int64)
    c_lengths = np.asarray(inputs["c_lengths"]).astype(np.int64)
    # bias is mathematically irrelevant: it adds (o_c@b) per i-row before
    # softmax over j, which softmax cancels exactly.

    from concourse.bass_utils import run_bass_kernel_spmd

    slots = _plan(q_lengths, c_lengths)
    in_maps = _host_inputs(o_c, o_q, W, q_lengths, slots)
    nc = _build_program(slots)

    trace = bool(int(os.environ.get("KERNEL_TRACE", "0")))
    res = run_bass_kernel_spmd(
        nc, in_maps, core_ids=list(range(N_CORES)), trace=trace
    )
    if trace:
        kernel.last_results = res

    out = np.zeros((B, Tc, H), dtype=np.float32)
    for s, sl in enumerate(slots):
        for core, g in enumerate(sl["batches"]):
            cl = int(c_lengths[g])
            out[g, :cl] = res.results[core][f"out{s}"][:cl].astype(np.float32)
    return out
